# revision 6
# baseline (speedup 1.0000x reference)
"""Trainium2 Bass kernel for nn_ReaReaConv (GCN-style message passing with
dynamic edge gating).

Math (per batch b):
    deg[n]   = in-degree(n) + 1 (self loop);  dis = rsqrt(deg)
    f_e      = keep*fdo + (1-keep)*(1-fdo), keep = sigmoid(2*flux[src]*flux[tgt])
    out[t]   = dis_t * ( (T-V)[t] @ Wc^T + V[t] @ Wd^T ) + bias
    T[t]     = sum_{e->t} dis_src * x[src_e]          (self loop: f=0 edge)
    V[t]     = sum_{e->t} dis_src * f_e * x[src_e]

Sharding: each of the 8 cores owns N/8 target nodes (tiles of 125). Host sorts
edges by target tile (indices/layout only; all FP math runs on device). Per
128-edge chunk the device gathers x rows (both batches interleaved, 512B) with
dma_gather, builds a dis_src-scaled one-hot (iota==tgt_local)*g with one fused
tensor_scalar, and accumulates T/V with two PE matmuls into PSUM. Final per
node tile: U = T-V, two small matmuls apply Wc/Wd, scale by dis_tgt, add bias.
"""

import os
from dataclasses import dataclass

import numpy as np

# -------------------- problem constants --------------------
N_NODES = 50000
N_EDGES = 1600000
BATCH = 2
C = 64
N_CORES = 8
TILE = 125           # target nodes per tile (one-hot width)
CHUNK = 128          # edges per matmul chunk (PE contraction)
SPLIT = 32768        # gather-table split (int16 signed index limit)
SELF_FLUX = 30.0     # sigmoid(2*30*30)==1.0 -> f==0 for self-loop edges


@dataclass(frozen=True)
class Cfg:
    n_nodes: int
    n_cores: int
    tile: int
    split: int
    capa: int  # chunks per tile from table A (src < split)
    capb: int  # chunks per tile from table B

    @property
    def nodes_per_core(self):
        return self.n_nodes // self.n_cores

    @property
    def ntc(self):  # tiles per core
        return self.nodes_per_core // self.tile

    @property
    def ct(self):
        return self.capa + self.capb

    @property
    def na(self):
        return min(self.split, self.n_nodes)

    @property
    def nb(self):
        return self.n_nodes - self.na


# -------------------- host prep (indices / layout only) --------------------

def _wrap16(idx_flat):
    """dma_gather index layout: [128, n/16] int16, idx[p, s] = flat[s*16+p],
    replicated across the 8 gpsimd cores (partition blocks of 16)."""
    n = len(idx_flat)
    assert n % 16 == 0
    w = np.asarray(idx_flat, np.int16).reshape(n // 16, 16).T  # [16, n/16]
    return np.tile(w, (8, 1))  # [128, n/16]


def prep(x, edge_index, f_disc_orig, fluxes, cfg: Cfg):
    """Returns (shared dict, list of per-core dicts). Integer/index/layout
    work only — no floating-point arithmetic."""
    n = cfg.n_nodes
    src0 = np.asarray(edge_index[0]).astype(np.int64)
    tgt0 = np.asarray(edge_index[1]).astype(np.int64)
    x = np.asarray(x, np.float32)
    fdo_in = np.asarray(f_disc_orig, np.float32)
    fluxes = np.asarray(fluxes, np.float32)

    deg = (np.bincount(tgt0, minlength=n) + 1).astype(np.float32)  # int-valued

    loops = np.arange(n, dtype=np.int64)
    src_all = np.concatenate([src0, loops])
    tgt_all = np.concatenate([tgt0, loops])
    fdo_all = np.concatenate([fdo_in, np.zeros(n, np.float32)])
    sf = np.full(n, SELF_FLUX, np.float32)
    fs0_all = np.concatenate([fluxes[0][src0], sf])
    fs1_all = np.concatenate([fluxes[1][src0], sf])
    ft0_all = np.concatenate([fluxes[0][tgt0], sf])
    ft1_all = np.concatenate([fluxes[1][tgt0], sf])
    degs_all = deg[src_all]

    perm = np.argsort(tgt_all, kind="stable")
    src_s = src_all[perm]
    tgt_s = tgt_all[perm]
    per_edge = np.stack(
        [fdo_all[perm], fs0_all[perm], fs1_all[perm], ft0_all[perm],
         ft1_all[perm], degs_all[perm]]
    )  # [6, E+N]

    tile_starts = np.searchsorted(tgt_s, np.arange(0, n + 1, cfg.tile))
    is_a = src_s < cfg.split

    ct = cfg.ct
    ntc = cfg.ntc
    shared = {
        # gather tables: row n = [x[0,n,:], x[1,n,:]]  (pure interleave)
        "xpa": np.ascontiguousarray(
            np.concatenate([x[0, : cfg.na], x[1, : cfg.na]], axis=1)),
        "xpb": np.ascontiguousarray(
            np.concatenate([x[0, cfg.na:], x[1, cfg.na:]], axis=1)),
        "iota": np.tile(np.arange(cfg.tile, dtype=np.float32), (128, 1)),
    }

    cores = []
    for core in range(cfg.n_cores):
        meta = np.zeros((128, ntc * 7 * ct), np.float32)
        idx16 = np.zeros((128, ntc * ct * 8), np.int16)
        degown = np.ones((128, ntc), np.float32)
        for tt in range(ntc):
            t = core * ntc + tt
            t0 = t * cfg.tile
            s, e = tile_starts[t], tile_starts[t + 1]
            sel_a = np.nonzero(is_a[s:e])[0] + s
            sel_b = np.nonzero(~is_a[s:e])[0] + s
            nA, nB = len(sel_a), len(sel_b)
            assert nA <= cfg.capa * CHUNK, (t, nA, cfg.capa * CHUNK)
            assert nB <= cfg.capb * CHUNK, (t, nB, cfg.capb * CHUNK)

            ids = np.zeros(ct * CHUNK, np.int64)
            tl = np.full(ct * CHUNK, -1.0, np.float32)
            pe = np.zeros((6, ct * CHUNK), np.float32)
            pe[5] = 1.0  # pad deg_src = 1
            off = cfg.capa * CHUNK
            ids[:nA] = src_s[sel_a]
            ids[off:off + nB] = src_s[sel_b] - cfg.na
            tl[:nA] = tgt_s[sel_a] - t0
            tl[off:off + nB] = tgt_s[sel_b] - t0
            pe[:, :nA] = per_edge[:, sel_a]
            pe[:, off:off + nB] = per_edge[:, sel_b]

            # meta blocks per tile: [tl, fs0, fs1, ft0, ft1, degs, fdo] x CT
            # cols; element (p, c) = edge[c*128+p]
            def ccols(v):
                return v.reshape(ct, CHUNK).T  # [128, ct]

            mslice = meta[:, tt * 7 * ct:(tt + 1) * 7 * ct]
            mslice[:, 0 * ct:1 * ct] = ccols(tl)
            mslice[:, 1 * ct:2 * ct] = ccols(pe[1])  # fs0
            mslice[:, 2 * ct:3 * ct] = ccols(pe[2])  # fs1
            mslice[:, 3 * ct:4 * ct] = ccols(pe[3])  # ft0
            mslice[:, 4 * ct:5 * ct] = ccols(pe[4])  # ft1
            mslice[:, 5 * ct:6 * ct] = ccols(pe[5])  # deg_src
            mslice[:, 6 * ct:7 * ct] = ccols(pe[0])  # fdo

            islice = idx16[:, tt * ct * 8:(tt + 1) * ct * 8]
            islice[:, : cfg.capa * 8] = _wrap16(ids[:off])
            islice[:, cfg.capa * 8:] = _wrap16(ids[off:])

            degown[:cfg.tile, tt] = deg[t0:t0 + cfg.tile]
        cores.append({"meta": meta, "idx16": idx16, "degown": degown})
    return shared, cores


# -------------------- device program --------------------

def build_nc(cfg: Cfg, W_conc, W_disc, bias):
    import concourse.bass as bass
    import concourse.tile as tile
    from concourse import bacc, mybir

    dt = mybir.dt
    act = mybir.ActivationFunctionType
    alu = mybir.AluOpType

    ct, capa, capb = cfg.ct, cfg.capa, cfg.capb
    ntc, T = cfg.ntc, cfg.tile

    nc = bacc.Bacc("TRN2", target_bir_lowering=False, debug=False)

    xpa = nc.dram_tensor("xpa", [cfg.na, 2 * C], dt.float32, kind="ExternalInput")
    xpb = nc.dram_tensor("xpb", [cfg.nb, 2 * C], dt.float32, kind="ExternalInput")
    meta = nc.dram_tensor("meta", [128, ntc * 7 * ct], dt.float32, kind="ExternalInput")
    idx16 = nc.dram_tensor("idx16", [128, ntc * ct * 8], dt.int16, kind="ExternalInput")
    degown = nc.dram_tensor("degown", [128, ntc], dt.float32, kind="ExternalInput")
    iota_d = nc.dram_tensor("iota", [128, T], dt.float32, kind="ExternalInput")
    wct_d = nc.dram_tensor("wct2", [128, C], dt.float32, kind="ExternalInput")
    wdt_d = nc.dram_tensor("wdt2", [128, C], dt.float32, kind="ExternalInput")
    bias_d = nc.dram_tensor("biasr", [128, C], dt.float32, kind="ExternalInput")
    out0 = nc.dram_tensor("out0", [ntc * T, C], dt.float32, kind="ExternalOutput")
    out1 = nc.dram_tensor("out1", [ntc * T, C], dt.float32, kind="ExternalOutput")
    outs = [out0, out1]

    with tile.TileContext(nc) as tc:
        with (
            tc.tile_pool(name="const", bufs=1) as constp,
            tc.tile_pool(name="meta", bufs=2) as metap,
            tc.tile_pool(name="idx", bufs=2) as idxp,
            tc.tile_pool(name="pp", bufs=2) as ppp,
            tc.tile_pool(name="xg", bufs=2) as xgp,
            tc.tile_pool(name="wv", bufs=2) as wvp,
            tc.tile_pool(name="og", bufs=6) as ogp,
            tc.tile_pool(name="uv", bufs=2) as uvp,
            tc.tile_pool(name="outp", bufs=2) as outsp,
            tc.tile_pool(name="ps_tv", bufs=2, space="PSUM") as pstv,
            tc.tile_pool(name="ps_o", bufs=2, space="PSUM") as pso,
        ):
            iota_sb = constp.tile([128, T], dt.float32)
            nc.sync.dma_start(iota_sb[:], iota_d[:, :])
            bias_sb = constp.tile([128, C], dt.float32)
            nc.sync.dma_start(bias_sb[:], bias_d[:, :])
            wct_sb = constp.tile([128, C], dt.float32)
            nc.sync.dma_start(wct_sb[:], wct_d[:, :])
            wdt_sb = constp.tile([128, C], dt.float32)
            nc.sync.dma_start(wdt_sb[:], wdt_d[:, :])
            degown_sb = constp.tile([128, ntc], dt.float32)
            nc.sync.dma_start(degown_sb[:], degown[:, :])

            for tt in range(ntc):
                meta_sb = metap.tile([128, 7 * ct], dt.float32)
                nc.sync.dma_start(meta_sb[:], meta[:, tt * 7 * ct:(tt + 1) * 7 * ct])
                idx_sb = idxp.tile([128, ct * 8], dt.int16)
                nc.sync.dma_start(idx_sb[:], idx16[:, tt * ct * 8:(tt + 1) * ct * 8])

                tl = meta_sb[:, 0 * ct:1 * ct]
                fs0 = meta_sb[:, 1 * ct:2 * ct]
                fs1 = meta_sb[:, 2 * ct:3 * ct]
                ft0 = meta_sb[:, 3 * ct:4 * ct]
                ft1 = meta_sb[:, 4 * ct:5 * ct]
                degs = meta_sb[:, 5 * ct:6 * ct]
                fdo = meta_sb[:, 6 * ct:7 * ct]

                # pre-pass: g = rsqrt(deg_src); f_b = keep_b*(2fdo-1) + (1-fdo)
                g_t = ppp.tile([128, ct], dt.float32, tag="g")
                nc.vector.reciprocal(g_t[:], degs)
                nc.scalar.activation(g_t[:], g_t[:], act.Sqrt)
                c1 = ppp.tile([128, ct], dt.float32, tag="c1")
                nc.vector.tensor_scalar(c1[:], fdo, 2.0, -1.0, alu.mult, alu.add)
                c0 = ppp.tile([128, ct], dt.float32, tag="c0")
                nc.vector.tensor_scalar(c0[:], fdo, -1.0, 1.0, alu.mult, alu.add)
                fts = []
                for bi, (fsx, ftx) in enumerate(((fs0, ft0), (fs1, ft1))):
                    prod = ppp.tile([128, ct], dt.float32, tag=f"prod{bi}")
                    nc.vector.tensor_mul(prod[:], fsx, ftx)
                    keep = ppp.tile([128, ct], dt.float32, tag=f"keep{bi}")
                    nc.scalar.activation(keep[:], prod[:], act.Sigmoid, scale=2.0)
                    fb = ppp.tile([128, ct], dt.float32, tag=f"f{bi}")
                    nc.vector.tensor_mul(fb[:], keep[:], c1[:])
                    nc.vector.tensor_add(fb[:], fb[:], c0[:])
                    fts.append(fb)

                # gathers: both batches per row (512B rows)
                xga = xgp.tile([128, capa * 2 * C], dt.float32, tag="xga")
                nc.gpsimd.dma_gather(
                    xga[:].rearrange("p (c r) -> p c r", r=2 * C),
                    xpa[:, :],
                    idx_sb[:, : capa * 8],
                    capa * CHUNK,
                    capa * CHUNK,
                    2 * C,
                    single_packet=False,
                )
                xgb = xgp.tile([128, capb * 2 * C], dt.float32, tag="xgb")
                nc.gpsimd.dma_gather(
                    xgb[:].rearrange("p (c r) -> p c r", r=2 * C),
                    xpb[:, :],
                    idx_sb[:, capa * 8:],
                    capb * CHUNK,
                    capb * CHUNK,
                    2 * C,
                    single_packet=False,
                )

                # w_V = f-scaled gathered rows (merged over chunks per segment)
                wva = wvp.tile([128, capa * 2 * C], dt.float32, tag="wva")
                wvb = wvp.tile([128, capb * 2 * C], dt.float32, tag="wvb")
                for (w3, x3, nch, foff) in (
                    (wva, xga, capa, 0),
                    (wvb, xgb, capb, capa),
                ):
                    wv3 = w3[:].rearrange("p (c r) -> p c r", r=2 * C)
                    xg3 = x3[:].rearrange("p (c r) -> p c r", r=2 * C)
                    for bi in range(2):
                        fcols = fts[bi][:, foff:foff + nch]
                        nc.vector.tensor_tensor(
                            wv3[:, :, bi * C:(bi + 1) * C],
                            xg3[:, :, bi * C:(bi + 1) * C],
                            fcols.unsqueeze(2).to_broadcast([128, nch, C]),
                            alu.mult,
                        )

                # chunk loop: one-hot + 2 matmuls accumulating T,V in PSUM
                t_ps = pstv.tile([128, T], dt.float32, tag="t_ps")
                v_ps = pstv.tile([128, T], dt.float32, tag="v_ps")
                for c in range(ct):
                    o_t = ogp.tile([128, T], dt.float32, tag="og")
                    nc.vector.tensor_scalar(
                        o_t[:], iota_sb[:],
                        tl[:, c:c + 1], g_t[:, c:c + 1],
                        alu.is_equal, alu.mult,
                    )
                    if c < capa:
                        xsl = xga[:, c * 2 * C:(c + 1) * 2 * C]
                        wsl = wva[:, c * 2 * C:(c + 1) * 2 * C]
                    else:
                        cc = c - capa
                        xsl = xgb[:, cc * 2 * C:(cc + 1) * 2 * C]
                        wsl = wvb[:, cc * 2 * C:(cc + 1) * 2 * C]
                    nc.tensor.matmul(
                        out=t_ps[:], lhsT=xsl, rhs=o_t[:],
                        start=(c == 0), stop=(c == ct - 1),
                    )
                    nc.tensor.matmul(
                        out=v_ps[:], lhsT=wsl, rhs=o_t[:],
                        start=(c == 0), stop=(c == ct - 1),
                    )

                # tile epilogue
                dis_sb = ppp.tile([128, 1], dt.float32, tag="dis")
                nc.vector.reciprocal(dis_sb[:], degown_sb[:, tt:tt + 1])
                nc.scalar.activation(dis_sb[:], dis_sb[:], act.Sqrt)

                vm = uvp.tile([128, T], dt.float32, tag="vm")
                nc.vector.tensor_copy(out=vm[:], in_=v_ps[:])
                um = uvp.tile([128, T], dt.float32, tag="um")
                nc.vector.tensor_tensor(um[:], t_ps[:], vm[:], alu.subtract)

                for bi in range(2):
                    rows = slice(64 * bi, 64 * bi + 64)
                    op_ps = pso.tile([T, C], dt.float32, tag=f"op{bi}")
                    nc.tensor.matmul(
                        out=op_ps[:], lhsT=um[rows, :], rhs=wct_sb[rows, :],
                        start=True, stop=False,
                    )
                    nc.tensor.matmul(
                        out=op_ps[:], lhsT=vm[rows, :], rhs=wdt_sb[rows, :],
                        start=False, stop=True,
                    )
                    o_sb = outsp.tile([128, C], dt.float32, tag=f"os{bi}")
                    nc.vector.tensor_scalar(
                        o_sb[:T, :], op_ps[:], dis_sb[:T, 0:1], None, alu.mult)
                    nc.vector.tensor_add(o_sb[:T, :], o_sb[:T, :], bias_sb[:T, :])
                    nc.sync.dma_start(
                        outs[bi][tt * T:(tt + 1) * T, :], o_sb[:T, :])

    nc.compile()
    return nc


def _shared_weights(W_conc, W_disc, bias):
    wct2 = np.zeros((128, C), np.float32)
    wdt2 = np.zeros((128, C), np.float32)
    wct2[:64] = np.asarray(W_conc, np.float32).T  # WcT[i, o] = Wc[o, i]
    wct2[64:] = wct2[:64]
    wdt2[:64] = np.asarray(W_disc, np.float32).T
    wdt2[64:] = wdt2[:64]
    biasr = np.tile(np.asarray(bias, np.float32)[None, :], (128, 1))
    return wct2, wdt2, biasr


_NC_CACHE = {}


def _caps_needed(edge_index, n, n_cores, tile, split):
    """Max per-tile chunk counts for the A/B table split (self loops incl.)."""
    src0 = np.asarray(edge_index[0]).astype(np.int64)
    tgt0 = np.asarray(edge_index[1]).astype(np.int64)
    loops = np.arange(n, dtype=np.int64)
    src_all = np.concatenate([src0, loops])
    tgt_all = np.concatenate([tgt0, loops])
    order = np.argsort(tgt_all, kind="stable")
    tgt_s, src_s = tgt_all[order], src_all[order]
    starts = np.searchsorted(tgt_s, np.arange(0, n + 1, tile))
    na = np.add.reduceat((src_s < split).astype(np.int64), starts[:-1])
    tot = np.diff(starts)
    maxa = int(na.max())
    maxb = int((tot - na).max())
    return -(-maxa // CHUNK), -(-maxb // CHUNK)


def _make_in_maps(x, edge_index, f_disc_orig, fluxes, W_conc, W_disc, bias,
                  cfg):
    shared, cores = prep(x, edge_index, f_disc_orig, fluxes, cfg)
    wct2, wdt2, biasr = _shared_weights(W_conc, W_disc, bias)
    in_maps = []
    for core in range(cfg.n_cores):
        m = dict(shared)
        m.update(cores[core])
        m["wct2"] = wct2
        m["wdt2"] = wdt2
        m["biasr"] = biasr
        in_maps.append(m)
    return in_maps


def _run(inputs, trace=False):
    from concourse.bass_utils import run_bass_kernel_spmd

    x = np.asarray(inputs["x"], np.float32)
    n = x.shape[1]
    capa, capb = _caps_needed(inputs["edge_index"], n, N_CORES, TILE, SPLIT)
    cfg = Cfg(n_nodes=n, n_cores=N_CORES, tile=TILE, split=SPLIT,
              capa=max(capa, 23), capb=max(capb, 13))
    in_maps = _make_in_maps(
        x, inputs["edge_index"], inputs["f_disc_orig"], inputs["fluxes"],
        inputs["W_conc"], inputs["W_disc"], inputs["bias"], cfg)

    if cfg not in _NC_CACHE:
        _NC_CACHE[cfg] = build_nc(cfg, None, None, None)
    nc = _NC_CACHE[cfg]

    res = run_bass_kernel_spmd(nc, in_maps, list(range(cfg.n_cores)),
                               trace=trace)
    out = np.zeros((BATCH, n, C), np.float32)
    npc = cfg.nodes_per_core
    for core in range(cfg.n_cores):
        out[0, core * npc:(core + 1) * npc] = res.results[core]["out0"]
        out[1, core * npc:(core + 1) * npc] = res.results[core]["out1"]
    return out, res


def kernel(x, edge_index, f_disc_orig, fluxes, W_conc, W_disc, bias):
    out, _ = _run(dict(x=x, edge_index=edge_index, f_disc_orig=f_disc_orig,
                       fluxes=fluxes, W_conc=W_conc, W_disc=W_disc, bias=bias))
    return out


def profile_run(inputs):
    out, res = _run(inputs, trace=True)
    return res.exec_time_ns


# revision 14
# speedup vs baseline: 1.0216x; 1.0216x over previous
"""Trainium2 Bass kernel for nn_ReaReaConv (GCN-style message passing with
dynamic edge gating).

Math (per batch b):
    deg[n]   = in-degree(n) + 1 (self loop);  dis = rsqrt(deg)
    f_e      = keep*fdo + (1-keep)*(1-fdo), keep = sigmoid(2*flux[src]*flux[tgt])
    out[t]   = dis_t * ( (T-V)[t] @ Wc^T + V[t] @ Wd^T ) + bias
    T[t]     = sum_{e->t} dis_src * x[src_e]          (self loop: f=0 edge)
    V[t]     = sum_{e->t} dis_src * f_e * x[src_e]

Sharding: each of the 8 cores owns N/8 target nodes (tiles of 125). Host sorts
edges by target tile (indices/layout only; all FP math runs on device).

Device phases:
 1. dis = rsqrt(deg) densely; build xp[n] = [dis_n*x[0,n], dis_n*x[1,n]]
    (the dis_src-prescaled gather table, 512B rows, both batches).
 2. Whole-core prepass computes per-edge f0/f1 from flux/fdo metadata.
 3. Per 125-node tile: dma_gather the tile's edges' xp rows (A/B table split
    for int16 indices), one merged is_equal builds all chunk one-hots, merged
    multiplies build f-scaled V-weights, then 2 PE matmuls per 128-edge chunk
    accumulate T/V in PSUM. Epilogue: U=T-V, project with Wc/Wd, scale by
    dis_tgt, add bias, store densely.
"""

from dataclasses import dataclass

import numpy as np

# -------------------- problem constants --------------------
N_NODES = 50000
N_EDGES = 1600000
BATCH = 2
C = 64
N_CORES = 8
TILE = 125           # target nodes per tile (one-hot width)
CHUNK = 128          # edges per matmul chunk (PE contraction)
SPLIT = 32768        # gather-table split (int16 signed index limit)
SELF_FLUX = 30.0     # sigmoid(2*30*30)==1.0 -> f==0 for self-loop edges


@dataclass(frozen=True)
class Cfg:
    n_nodes: int
    n_cores: int
    tile: int
    split: int
    capa: int  # chunks per tile from table A (src < split)
    capb: int  # chunks per tile from table B

    @property
    def nodes_per_core(self):
        return self.n_nodes // self.n_cores

    @property
    def ntc(self):  # tiles per core
        return self.nodes_per_core // self.tile

    @property
    def ct(self):
        return self.capa + self.capb

    @property
    def ctn(self):
        return self.ntc * self.ct

    @property
    def na(self):
        return min(self.split, self.n_nodes)

    @property
    def nb(self):
        return self.n_nodes - self.na

    @property
    def nblk(self):
        return -(-self.n_nodes // 128)


# -------------------- host prep (indices / layout only) --------------------

def _wrap16(idx_flat):
    """dma_gather index layout: [128, n/16] int16, idx[p, s] = flat[s*16+p],
    replicated across the 8 gpsimd cores (partition blocks of 16)."""
    n = len(idx_flat)
    assert n % 16 == 0
    w = np.asarray(idx_flat, np.int16).reshape(n // 16, 16).T  # [16, n/16]
    return np.tile(w, (8, 1))  # [128, n/16]


def prep(x, edge_index, f_disc_orig, fluxes, cfg: Cfg):
    """Returns (shared dict, list of per-core dicts). Integer/index/layout
    work only — no floating-point arithmetic."""
    n = cfg.n_nodes
    src0 = np.asarray(edge_index[0]).astype(np.int64)
    tgt0 = np.asarray(edge_index[1]).astype(np.int64)
    x = np.asarray(x, np.float32)
    fdo_in = np.asarray(f_disc_orig, np.float32)
    fluxes = np.asarray(fluxes, np.float32)

    deg = (np.bincount(tgt0, minlength=n) + 1).astype(np.float32)  # int-valued

    loops = np.arange(n, dtype=np.int64)
    src_all = np.concatenate([src0, loops])
    tgt_all = np.concatenate([tgt0, loops])
    sf = np.full(n, SELF_FLUX, np.float32)
    per_edge_all = np.stack([
        np.concatenate([fdo_in, np.zeros(n, np.float32)]),
        np.concatenate([fluxes[0][src0], sf]),
        np.concatenate([fluxes[1][src0], sf]),
        np.concatenate([fluxes[0][tgt0], sf]),
        np.concatenate([fluxes[1][tgt0], sf]),
    ])  # [5, E+N]: fdo, fs0, fs1, ft0, ft1

    perm = np.argsort(tgt_all, kind="stable")
    src_s = src_all[perm]
    tgt_s = tgt_all[perm]
    per_edge = per_edge_all[:, perm]

    tile_starts = np.searchsorted(tgt_s, np.arange(0, n + 1, cfg.tile))
    is_a = src_s < cfg.split

    ct, ntc, ctn = cfg.ct, cfg.ntc, cfg.ctn

    degflat = np.ones(cfg.nblk * 128, np.float32)
    degflat[:n] = deg
    # layout: degall[p, blk] = deg[blk*128 + p]
    degall = np.ascontiguousarray(degflat.reshape(cfg.nblk, 128).T)

    shared = {
        "x0d": np.ascontiguousarray(x[0]),
        "x1d": np.ascontiguousarray(x[1]),
        "degall": degall,
        "iota": np.tile(np.arange(cfg.tile, dtype=np.float32), (128, 1)),
    }

    names = ["fdo", "fs0", "fs1", "ft0", "ft1"]
    cores = []
    for core in range(cfg.n_cores):
        tl_all = np.full((128, ctn), -1.0, np.float32)
        pe_all = np.zeros((5, 128, ctn), np.float32)
        idx16 = np.zeros((128, ctn * 8), np.int16)
        degown = np.ones((128, ntc), np.float32)
        for tt in range(ntc):
            t = core * ntc + tt
            t0 = t * cfg.tile
            s, e = tile_starts[t], tile_starts[t + 1]
            sel_a = np.nonzero(is_a[s:e])[0] + s
            sel_b = np.nonzero(~is_a[s:e])[0] + s
            nA, nB = len(sel_a), len(sel_b)
            assert nA <= cfg.capa * CHUNK, (t, nA, cfg.capa * CHUNK)
            assert nB <= cfg.capb * CHUNK, (t, nB, cfg.capb * CHUNK)

            ids = np.zeros(ct * CHUNK, np.int64)
            tl = np.full(ct * CHUNK, -1.0, np.float32)
            pe = np.zeros((5, ct * CHUNK), np.float32)
            off = cfg.capa * CHUNK
            ids[:nA] = src_s[sel_a]
            ids[off:off + nB] = src_s[sel_b] - cfg.na
            tl[:nA] = tgt_s[sel_a] - t0
            tl[off:off + nB] = tgt_s[sel_b] - t0
            pe[:, :nA] = per_edge[:, sel_a]
            pe[:, off:off + nB] = per_edge[:, sel_b]

            # chunk-transposed layout: element (p, c) = edge[c*128+p]
            cols = slice(tt * ct, (tt + 1) * ct)
            tl_all[:, cols] = tl.reshape(ct, CHUNK).T
            for j in range(5):
                pe_all[j][:, cols] = pe[j].reshape(ct, CHUNK).T

            islice = idx16[:, tt * ct * 8:(tt + 1) * ct * 8]
            islice[:, : cfg.capa * 8] = _wrap16(ids[:off])
            islice[:, cfg.capa * 8:] = _wrap16(ids[off:])

            degown[:cfg.tile, tt] = deg[t0:t0 + cfg.tile]
        d = {"tl": tl_all, "idx16": idx16, "degown": degown}
        for j, nm in enumerate(names):
            d[nm] = np.ascontiguousarray(pe_all[j])
        cores.append(d)
    return shared, cores


# -------------------- device program --------------------

def build_nc(cfg: Cfg):
    import concourse.bass as bass
    import concourse.tile as tile
    from concourse import bacc, mybir

    dt = mybir.dt
    act = mybir.ActivationFunctionType
    alu = mybir.AluOpType

    ct, capa, capb = cfg.ct, cfg.capa, cfg.capb
    ntc, T, ctn = cfg.ntc, cfg.tile, cfg.ctn
    n, nblk = cfg.n_nodes, cfg.nblk

    nc = bacc.Bacc("TRN2", target_bir_lowering=False, debug=False)

    x0d = nc.dram_tensor("x0d", [n, C], dt.float32, kind="ExternalInput")
    x1d = nc.dram_tensor("x1d", [n, C], dt.float32, kind="ExternalInput")
    degall_d = nc.dram_tensor("degall", [128, nblk], dt.float32, kind="ExternalInput")
    tl_d = nc.dram_tensor("tl", [128, ctn], dt.float32, kind="ExternalInput")
    fdo_d = nc.dram_tensor("fdo", [128, ctn], dt.float32, kind="ExternalInput")
    fs0_d = nc.dram_tensor("fs0", [128, ctn], dt.float32, kind="ExternalInput")
    fs1_d = nc.dram_tensor("fs1", [128, ctn], dt.float32, kind="ExternalInput")
    ft0_d = nc.dram_tensor("ft0", [128, ctn], dt.float32, kind="ExternalInput")
    ft1_d = nc.dram_tensor("ft1", [128, ctn], dt.float32, kind="ExternalInput")
    idx16_d = nc.dram_tensor("idx16", [128, ctn * 8], dt.int16, kind="ExternalInput")
    degown_d = nc.dram_tensor("degown", [128, ntc], dt.float32, kind="ExternalInput")
    iota_d = nc.dram_tensor("iota", [128, T], dt.float32, kind="ExternalInput")
    wct_d = nc.dram_tensor("wct2", [128, C], dt.float32, kind="ExternalInput")
    wdt_d = nc.dram_tensor("wdt2", [128, C], dt.float32, kind="ExternalInput")
    bias_d = nc.dram_tensor("biasr", [128, C], dt.float32, kind="ExternalInput")
    out0 = nc.dram_tensor("out0", [ntc * T, C], dt.float32, kind="ExternalOutput")
    out1 = nc.dram_tensor("out1", [ntc * T, C], dt.float32, kind="ExternalOutput")
    outs = [out0, out1]

    with tile.TileContext(nc) as tc:
        with (
            tc.tile_pool(name="const", bufs=1) as constp,
            tc.tile_pool(name="res", bufs=1) as resp,
            tc.tile_pool(name="xpd", bufs=1, space="DRAM") as xpdp,
        ):
            xp = xpdp.tile([n, 2 * C], dt.float32)  # prescaled gather table
            iota_sb = constp.tile([128, T], dt.float32)
            nc.sync.dma_start(iota_sb[:], iota_d[:, :])
            bias_sb = constp.tile([128, C], dt.float32)
            nc.sync.dma_start(bias_sb[:], bias_d[:, :])
            wct_sb = constp.tile([128, C], dt.float32)
            nc.sync.dma_start(wct_sb[:], wct_d[:, :])
            wdt_sb = constp.tile([128, C], dt.float32)
            nc.sync.dma_start(wdt_sb[:], wdt_d[:, :])

            # resident per-core data
            tl_sb = resp.tile([128, ctn], dt.float32)
            nc.sync.dma_start(tl_sb[:], tl_d[:, :])
            idx_sb = resp.tile([128, ctn * 8], dt.int16)
            nc.sync.dma_start(idx_sb[:], idx16_d[:, :])
            f_sb = [resp.tile([128, ctn], dt.float32, tag=f"f{b}", name=f"f{b}")
                    for b in range(2)]

            # dis for own target nodes
            disown_sb = resp.tile([128, ntc], dt.float32)
            nc.sync.dma_start(disown_sb[:], degown_d[:, :])
            nc.vector.reciprocal(disown_sb[:], disown_sb[:])
            nc.scalar.activation(disown_sb[:], disown_sb[:], act.Sqrt)

            # ---- phase 1: dis over all nodes + prescaled gather table xp ----
            with (
                tc.tile_pool(name="bld", bufs=3) as bld,
                tc.tile_pool(name="disp", bufs=1) as disp,
            ):
                dis_sb = disp.tile([128, nblk], dt.float32)
                nc.sync.dma_start(dis_sb[:], degall_d[:, :])
                nc.vector.reciprocal(dis_sb[:], dis_sb[:])
                nc.scalar.activation(dis_sb[:], dis_sb[:], act.Sqrt)

                SB = 8
                afull = n // 128  # full 128-row blocks
                tail_rows = n - afull * 128
                x0v = x0d[0:afull * 128, :].rearrange("(a p) i -> p a i", p=128)
                x1v = x1d[0:afull * 128, :].rearrange("(a p) i -> p a i", p=128)
                xpv = xp[0:afull * 128, :].rearrange("(a p) r -> p a r", p=128)
                if tail_rows:
                    x0t = x0d[afull * 128:n, :].unsqueeze(1)
                    x1t = x1d[afull * 128:n, :].unsqueeze(1)
                    xpt = xp[afull * 128:n, :].unsqueeze(1)
                spans = [(i * SB, min(SB, afull - i * SB), 128)
                         for i in range((afull + SB - 1) // SB)]
                if tail_rows:
                    spans.append((afull, 1, tail_rows))
                for (a0, na_, rows) in spans:
                    is_tail = rows < 128
                    src0v = x0t if is_tail else x0v[:, a0:a0 + na_]
                    src1v = x1t if is_tail else x1v[:, a0:a0 + na_]
                    dstv = xpt if is_tail else xpv[:, a0:a0 + na_]
                    xa = bld.tile([128, SB * C], dt.float32, tag="xa")
                    nc.sync.dma_start(
                        xa[:rows].rearrange("p (a i) -> p a i", i=C)[:, :na_],
                        src0v[:rows],
                    )
                    xb = bld.tile([128, SB * C], dt.float32, tag="xb")
                    nc.sync.dma_start(
                        xb[:rows].rearrange("p (a i) -> p a i", i=C)[:, :na_],
                        src1v[:rows],
                    )
                    xw = bld.tile([128, SB * 2 * C], dt.float32, tag="xw")
                    xw3 = xw[:rows].rearrange("p (a r) -> p a r", r=2 * C)
                    dcols = dis_sb[:rows, a0:a0 + na_].unsqueeze(2)
                    nc.vector.tensor_tensor(
                        xw3[:, :na_, 0:C],
                        xa[:rows].rearrange("p (a i) -> p a i", i=C)[:, :na_],
                        dcols.to_broadcast([rows, na_, C]),
                        alu.mult,
                    )
                    nc.vector.tensor_tensor(
                        xw3[:, :na_, C:2 * C],
                        xb[:rows].rearrange("p (a i) -> p a i", i=C)[:, :na_],
                        dcols.to_broadcast([rows, na_, C]),
                        alu.mult,
                    )
                    nc.sync.dma_start(dstv[:rows], xw3[:, :na_])

                # ---- phase 2: whole-core prepass f0/f1 ----
                with tc.tile_pool(name="pp", bufs=1) as ppp:
                    fdo_sb = ppp.tile([128, ctn], dt.float32)
                    nc.sync.dma_start(fdo_sb[:], fdo_d[:, :])
                    c1 = ppp.tile([128, ctn], dt.float32)
                    nc.vector.tensor_scalar(
                        c1[:], fdo_sb[:], 2.0, -1.0, alu.mult, alu.add)
                    c0 = ppp.tile([128, ctn], dt.float32)
                    nc.vector.tensor_scalar(
                        c0[:], fdo_sb[:], -1.0, 1.0, alu.mult, alu.add)
                    for b, (fsd, ftd) in enumerate(((fs0_d, ft0_d), (fs1_d, ft1_d))):
                        fs_sb = ppp.tile([128, ctn], dt.float32, tag="fs")
                        nc.sync.dma_start(fs_sb[:], fsd[:, :])
                        ft_sb = ppp.tile([128, ctn], dt.float32, tag="ft")
                        nc.sync.dma_start(ft_sb[:], ftd[:, :])
                        nc.vector.tensor_mul(fs_sb[:], fs_sb[:], ft_sb[:])
                        nc.scalar.activation(
                            ft_sb[:], fs_sb[:], act.Sigmoid, scale=2.0)
                        nc.vector.tensor_mul(f_sb[b][:], ft_sb[:], c1[:])
                        nc.vector.tensor_add(f_sb[b][:], f_sb[b][:], c0[:])

            # ---- phase 3: main loop over node tiles ----
            with (
                tc.tile_pool(name="xg", bufs=2) as xgp,
                tc.tile_pool(name="wv", bufs=2) as wvp,
                tc.tile_pool(name="og", bufs=2) as ogp,
                tc.tile_pool(name="uv", bufs=2) as uvp,
                tc.tile_pool(name="outp", bufs=2) as outsp,
                tc.tile_pool(name="ps_tv", bufs=2, space="PSUM") as pstv,
                tc.tile_pool(name="ps_o", bufs=2, space="PSUM") as pso,
            ):
                for tt in range(ntc):
                    xga = xgp.tile([128, capa * 2 * C], dt.float32, tag="xga")
                    ga = nc.gpsimd.dma_gather(
                        xga[:].rearrange("p (c r) -> p c r", r=2 * C),
                        xp[0:cfg.na, :],
                        idx_sb[:, tt * ct * 8: tt * ct * 8 + capa * 8],
                        capa * CHUNK, capa * CHUNK, 2 * C,
                        single_packet=False,
                    )
                    xgb = xgp.tile([128, capb * 2 * C], dt.float32, tag="xgb")
                    gb = nc.gpsimd.dma_gather(
                        xgb[:].rearrange("p (c r) -> p c r", r=2 * C),
                        xp[cfg.na:n, :],
                        idx_sb[:, tt * ct * 8 + capa * 8:(tt + 1) * ct * 8],
                        capb * CHUNK, capb * CHUNK, 2 * C,
                        single_packet=False,
                    )


                    # merged one-hot for all chunks of this tile
                    o_all = ogp.tile([128, ct * T], dt.float32, tag="og")
                    tl_cols = tl_sb[:, tt * ct:(tt + 1) * ct].unsqueeze(2)
                    nc.vector.tensor_tensor(
                        o_all[:].rearrange("p (c t) -> p c t", t=T),
                        tl_cols.to_broadcast([128, ct, T]),
                        iota_sb[:].unsqueeze(1).to_broadcast([128, ct, T]),
                        alu.is_equal,
                    )

                    # f-scaled V weights, merged per segment & batch-half
                    wva = wvp.tile([128, capa * 2 * C], dt.float32, tag="wva")
                    wvb = wvp.tile([128, capb * 2 * C], dt.float32, tag="wvb")
                    for (w3, x3, nch, foff) in (
                        (wva, xga, capa, tt * ct),
                        (wvb, xgb, capb, tt * ct + capa),
                    ):
                        wv3 = w3[:].rearrange("p (c r) -> p c r", r=2 * C)
                        xg3 = x3[:].rearrange("p (c r) -> p c r", r=2 * C)
                        for bi in range(2):
                            fcols = f_sb[bi][:, foff:foff + nch]
                            nc.vector.tensor_tensor(
                                wv3[:, :, bi * C:(bi + 1) * C],
                                xg3[:, :, bi * C:(bi + 1) * C],
                                fcols.unsqueeze(2).to_broadcast([128, nch, C]),
                                alu.mult,
                            )

                    t_ps = pstv.tile([128, T], dt.float32, tag="t_ps")
                    v_ps = pstv.tile([128, T], dt.float32, tag="v_ps")
                    for c in range(ct):
                        osl = o_all[:, c * T:(c + 1) * T]
                        if c < capa:
                            xsl = xga[:, c * 2 * C:(c + 1) * 2 * C]
                            wsl = wva[:, c * 2 * C:(c + 1) * 2 * C]
                        else:
                            cc = c - capa
                            xsl = xgb[:, cc * 2 * C:(cc + 1) * 2 * C]
                            wsl = wvb[:, cc * 2 * C:(cc + 1) * 2 * C]
                        nc.tensor.matmul(
                            out=t_ps[:], lhsT=xsl, rhs=osl,
                            start=(c == 0), stop=(c == ct - 1),
                        )
                        nc.tensor.matmul(
                            out=v_ps[:], lhsT=wsl, rhs=osl,
                            start=(c == 0), stop=(c == ct - 1),
                        )

                    # epilogue
                    vm = uvp.tile([128, T], dt.float32, tag="vm")
                    nc.vector.tensor_copy(out=vm[:], in_=v_ps[:])
                    um = uvp.tile([128, T], dt.float32, tag="um")
                    nc.vector.tensor_tensor(um[:], t_ps[:], vm[:], alu.subtract)

                    for bi in range(2):
                        rows = slice(64 * bi, 64 * bi + 64)
                        op_ps = pso.tile([T, C], dt.float32, tag=f"op{bi}")
                        nc.tensor.matmul(
                            out=op_ps[:], lhsT=um[rows, :], rhs=wct_sb[rows, :],
                            start=True, stop=False,
                        )
                        nc.tensor.matmul(
                            out=op_ps[:], lhsT=vm[rows, :], rhs=wdt_sb[rows, :],
                            start=False, stop=True,
                        )
                        o_sb = outsp.tile([128, C], dt.float32, tag=f"os{bi}")
                        nc.vector.tensor_scalar(
                            o_sb[:T, :], op_ps[:], disown_sb[:T, tt:tt + 1],
                            None, alu.mult)
                        nc.vector.tensor_add(
                            o_sb[:T, :], o_sb[:T, :], bias_sb[:T, :])
                        nc.sync.dma_start(
                            outs[bi][tt * T:(tt + 1) * T, :], o_sb[:T, :])

    nc.compile()
    return nc


def _shared_weights(W_conc, W_disc, bias):
    wct2 = np.zeros((128, C), np.float32)
    wdt2 = np.zeros((128, C), np.float32)
    wct2[:64] = np.asarray(W_conc, np.float32).T  # WcT[i, o] = Wc[o, i]
    wct2[64:] = wct2[:64]
    wdt2[:64] = np.asarray(W_disc, np.float32).T
    wdt2[64:] = wdt2[:64]
    biasr = np.tile(np.asarray(bias, np.float32)[None, :], (128, 1))
    return wct2, wdt2, biasr


_NC_CACHE = {}


def _caps_needed(edge_index, n, n_cores, tile, split):
    """Max per-tile chunk counts for the A/B table split (self loops incl.)."""
    src0 = np.asarray(edge_index[0]).astype(np.int64)
    tgt0 = np.asarray(edge_index[1]).astype(np.int64)
    loops = np.arange(n, dtype=np.int64)
    src_all = np.concatenate([src0, loops])
    tgt_all = np.concatenate([tgt0, loops])
    order = np.argsort(tgt_all, kind="stable")
    tgt_s, src_s = tgt_all[order], src_all[order]
    starts = np.searchsorted(tgt_s, np.arange(0, n + 1, tile))
    na = np.add.reduceat((src_s < split).astype(np.int64), starts[:-1])
    tot = np.diff(starts)
    maxa = int(na.max())
    maxb = int((tot - na).max())
    return -(-maxa // CHUNK), -(-maxb // CHUNK)


def _make_in_maps(x, edge_index, f_disc_orig, fluxes, W_conc, W_disc, bias,
                  cfg):
    shared, cores = prep(x, edge_index, f_disc_orig, fluxes, cfg)
    wct2, wdt2, biasr = _shared_weights(W_conc, W_disc, bias)
    in_maps = []
    for core in range(cfg.n_cores):
        m = dict(shared)
        m.update(cores[core])
        m["wct2"] = wct2
        m["wdt2"] = wdt2
        m["biasr"] = biasr
        in_maps.append(m)
    return in_maps


def _run(inputs, trace=False):
    from concourse.bass_utils import run_bass_kernel_spmd

    x = np.asarray(inputs["x"], np.float32)
    n = x.shape[1]
    capa, capb = _caps_needed(inputs["edge_index"], n, N_CORES, TILE, SPLIT)
    cfg = Cfg(n_nodes=n, n_cores=N_CORES, tile=TILE, split=SPLIT,
              capa=max(capa, 23), capb=max(capb, 13))
    in_maps = _make_in_maps(
        x, inputs["edge_index"], inputs["f_disc_orig"], inputs["fluxes"],
        inputs["W_conc"], inputs["W_disc"], inputs["bias"], cfg)

    if cfg not in _NC_CACHE:
        _NC_CACHE[cfg] = build_nc(cfg)
    nc = _NC_CACHE[cfg]

    res = run_bass_kernel_spmd(nc, in_maps, list(range(cfg.n_cores)),
                               trace=trace)
    out = np.zeros((BATCH, n, C), np.float32)
    npc = cfg.nodes_per_core
    for core in range(cfg.n_cores):
        out[0, core * npc:(core + 1) * npc] = res.results[core]["out0"]
        out[1, core * npc:(core + 1) * npc] = res.results[core]["out1"]
    return out, res


def kernel(x, edge_index, f_disc_orig, fluxes, W_conc, W_disc, bias):
    out, _ = _run(dict(x=x, edge_index=edge_index, f_disc_orig=f_disc_orig,
                       fluxes=fluxes, W_conc=W_conc, W_disc=W_disc, bias=bias))
    return out


def profile_run(inputs):
    out, res = _run(inputs, trace=True)
    return res.exec_time_ns


# revision 16
# speedup vs baseline: 1.8640x; 1.8245x over previous
"""Trainium2 Bass kernel for nn_ReaReaConv (GCN-style message passing with
dynamic edge gating).

Math (per batch b):
    deg[n]   = in-degree(n) + 1 (self loop);  dis = rsqrt(deg)
    f_e      = keep*fdo + (1-keep)*(1-fdo), keep = sigmoid(2*flux[src]*flux[tgt])
    out[t]   = dis_t * ( (T-V)[t] @ Wc^T + V[t] @ Wd^T ) + bias
    T[t]     = sum_{e->t} dis_src * x[src_e]          (self loop: f=0 edge)
    V[t]     = sum_{e->t} dis_src * f_e * x[src_e]

Sharding: each of the 8 cores owns N/8 target nodes (tiles of 125). Host sorts
edges by target tile (indices/layout only; all FP math runs on device).

Device phases:
 1. dis = rsqrt(deg) densely; build xp[n] = [dis_n*x[0,n], dis_n*x[1,n]]
    (the dis_src-prescaled gather table, 512B rows, both batches).
 2. Whole-core prepass computes per-edge f0/f1 from flux/fdo metadata.
 3. Per 125-node tile: dma_gather the tile's edges' xp rows (A/B table split
    for int16 indices), one merged is_equal builds all chunk one-hots, merged
    multiplies build f-scaled V-weights, then 2 PE matmuls per 128-edge chunk
    accumulate T/V in PSUM. Epilogue: U=T-V, project with Wc/Wd, scale by
    dis_tgt, add bias, store densely.
"""

from dataclasses import dataclass

import numpy as np

# -------------------- problem constants --------------------
N_NODES = 50000
N_EDGES = 1600000
BATCH = 2
C = 64
N_CORES = 8
TILE = 125           # target nodes per tile (one-hot width)
CHUNK = 128          # edges per matmul chunk (PE contraction)
SPLIT = 32768        # gather-table split (int16 signed index limit)
SELF_FLUX = 30.0     # sigmoid(2*30*30)==1.0 -> f==0 for self-loop edges


@dataclass(frozen=True)
class Cfg:
    n_nodes: int
    n_cores: int
    tile: int
    split: int
    capa: int  # chunks per tile from table A (src < split)
    capb: int  # chunks per tile from table B

    @property
    def nodes_per_core(self):
        return self.n_nodes // self.n_cores

    @property
    def ntc(self):  # tiles per core
        return self.nodes_per_core // self.tile

    @property
    def ct(self):
        return self.capa + self.capb

    @property
    def ctn(self):
        return self.ntc * self.ct

    @property
    def na(self):
        return min(self.split, self.n_nodes)

    @property
    def nb(self):
        return self.n_nodes - self.na

    @property
    def nblk(self):
        return -(-self.n_nodes // 128)


# -------------------- host prep (indices / layout only) --------------------

def _wrap16(idx_flat):
    """dma_gather index layout: [128, n/16] int16, idx[p, s] = flat[s*16+p],
    replicated across the 8 gpsimd cores (partition blocks of 16)."""
    n = len(idx_flat)
    assert n % 16 == 0
    w = np.asarray(idx_flat, np.int16).reshape(n // 16, 16).T  # [16, n/16]
    return np.tile(w, (8, 1))  # [128, n/16]


def prep(x, edge_index, f_disc_orig, fluxes, cfg: Cfg):
    """Returns (shared dict, list of per-core dicts). Integer/index/layout
    work only — no floating-point arithmetic."""
    n = cfg.n_nodes
    src0 = np.asarray(edge_index[0]).astype(np.int64)
    tgt0 = np.asarray(edge_index[1]).astype(np.int64)
    x = np.asarray(x, np.float32)
    fdo_in = np.asarray(f_disc_orig, np.float32)
    fluxes = np.asarray(fluxes, np.float32)

    deg = (np.bincount(tgt0, minlength=n) + 1).astype(np.float32)  # int-valued

    loops = np.arange(n, dtype=np.int64)
    src_all = np.concatenate([src0, loops])
    tgt_all = np.concatenate([tgt0, loops])
    sf = np.full(n, SELF_FLUX, np.float32)
    per_edge_all = np.stack([
        np.concatenate([fdo_in, np.zeros(n, np.float32)]),
        np.concatenate([fluxes[0][src0], sf]),
        np.concatenate([fluxes[1][src0], sf]),
        np.concatenate([fluxes[0][tgt0], sf]),
        np.concatenate([fluxes[1][tgt0], sf]),
        deg[np.concatenate([src0, loops])],  # deg at src end (int-valued)
    ])  # [6, E+N]: fdo, fs0, fs1, ft0, ft1, degs

    perm = np.argsort(tgt_all, kind="stable")
    src_s = src_all[perm]
    tgt_s = tgt_all[perm]
    per_edge = per_edge_all[:, perm]

    tile_starts = np.searchsorted(tgt_s, np.arange(0, n + 1, cfg.tile))
    is_a = src_s < cfg.split

    ct, ntc, ctn = cfg.ct, cfg.ntc, cfg.ctn

    shared = {
        # gather tables: row n = [x[0,n,:], x[1,n,:]]  (pure interleave)
        "xpa": np.ascontiguousarray(
            np.concatenate([x[0, : cfg.na], x[1, : cfg.na]], axis=1)),
        "xpb": np.ascontiguousarray(
            np.concatenate([x[0, cfg.na:], x[1, cfg.na:]], axis=1)),
        "iota": np.tile(np.arange(cfg.tile, dtype=np.float32), (128, 1)),
    }

    names = ["fdo", "fs0", "fs1", "ft0", "ft1", "degs"]
    cores = []
    for core in range(cfg.n_cores):
        tl_all = np.full((128, ctn), -1.0, np.float32)
        pe_all = np.zeros((6, 128, ctn), np.float32)
        pe_all[5] = 1.0  # pad deg_src = 1
        idx16 = np.zeros((128, ctn * 8), np.int16)
        degown = np.ones((128, ntc), np.float32)
        for tt in range(ntc):
            t = core * ntc + tt
            t0 = t * cfg.tile
            s, e = tile_starts[t], tile_starts[t + 1]
            sel_a = np.nonzero(is_a[s:e])[0] + s
            sel_b = np.nonzero(~is_a[s:e])[0] + s
            nA, nB = len(sel_a), len(sel_b)
            assert nA <= cfg.capa * CHUNK, (t, nA, cfg.capa * CHUNK)
            assert nB <= cfg.capb * CHUNK, (t, nB, cfg.capb * CHUNK)

            ids = np.zeros(ct * CHUNK, np.int64)
            tl = np.full(ct * CHUNK, -1.0, np.float32)
            pe = np.zeros((6, ct * CHUNK), np.float32)
            pe[5] = 1.0
            off = cfg.capa * CHUNK
            ids[:nA] = src_s[sel_a]
            ids[off:off + nB] = src_s[sel_b] - cfg.na
            tl[:nA] = tgt_s[sel_a] - t0
            tl[off:off + nB] = tgt_s[sel_b] - t0
            pe[:, :nA] = per_edge[:, sel_a]
            pe[:, off:off + nB] = per_edge[:, sel_b]

            # chunk-transposed layout: element (p, c) = edge[c*128+p]
            cols = slice(tt * ct, (tt + 1) * ct)
            tl_all[:, cols] = tl.reshape(ct, CHUNK).T
            for j in range(6):
                pe_all[j][:, cols] = pe[j].reshape(ct, CHUNK).T

            islice = idx16[:, tt * ct * 8:(tt + 1) * ct * 8]
            islice[:, : cfg.capa * 8] = _wrap16(ids[:off])
            islice[:, cfg.capa * 8:] = _wrap16(ids[off:])

            degown[:cfg.tile, tt] = deg[t0:t0 + cfg.tile]
        d = {"tl": tl_all, "idx16": idx16, "degown": degown}
        for j, nm in enumerate(names):
            d[nm] = np.ascontiguousarray(pe_all[j])
        cores.append(d)
    return shared, cores


# -------------------- device program --------------------

def build_nc(cfg: Cfg):
    import concourse.bass as bass
    import concourse.tile as tile
    from concourse import bacc, mybir

    dt = mybir.dt
    act = mybir.ActivationFunctionType
    alu = mybir.AluOpType

    ct, capa, capb = cfg.ct, cfg.capa, cfg.capb
    ntc, T, ctn = cfg.ntc, cfg.tile, cfg.ctn
    n = cfg.n_nodes

    nc = bacc.Bacc("TRN2", target_bir_lowering=False, debug=False,
                   num_swdge_queues=4)

    xpa = nc.dram_tensor("xpa", [cfg.na, 2 * C], dt.float32, kind="ExternalInput")
    xpb = nc.dram_tensor("xpb", [cfg.nb, 2 * C], dt.float32, kind="ExternalInput")
    tl_d = nc.dram_tensor("tl", [128, ctn], dt.float32, kind="ExternalInput")
    fdo_d = nc.dram_tensor("fdo", [128, ctn], dt.float32, kind="ExternalInput")
    fs0_d = nc.dram_tensor("fs0", [128, ctn], dt.float32, kind="ExternalInput")
    fs1_d = nc.dram_tensor("fs1", [128, ctn], dt.float32, kind="ExternalInput")
    ft0_d = nc.dram_tensor("ft0", [128, ctn], dt.float32, kind="ExternalInput")
    ft1_d = nc.dram_tensor("ft1", [128, ctn], dt.float32, kind="ExternalInput")
    degs_d = nc.dram_tensor("degs", [128, ctn], dt.float32, kind="ExternalInput")
    idx16_d = nc.dram_tensor("idx16", [128, ctn * 8], dt.int16, kind="ExternalInput")
    degown_d = nc.dram_tensor("degown", [128, ntc], dt.float32, kind="ExternalInput")
    iota_d = nc.dram_tensor("iota", [128, T], dt.float32, kind="ExternalInput")
    wct_d = nc.dram_tensor("wct2", [128, C], dt.float32, kind="ExternalInput")
    wdt_d = nc.dram_tensor("wdt2", [128, C], dt.float32, kind="ExternalInput")
    bias_d = nc.dram_tensor("biasr", [128, C], dt.float32, kind="ExternalInput")
    out0 = nc.dram_tensor("out0", [ntc * T, C], dt.float32, kind="ExternalOutput")
    out1 = nc.dram_tensor("out1", [ntc * T, C], dt.float32, kind="ExternalOutput")
    outs = [out0, out1]

    with tile.TileContext(nc) as tc:
        with (
            tc.tile_pool(name="const", bufs=1) as constp,
            tc.tile_pool(name="res", bufs=1) as resp,
        ):
            iota_sb = constp.tile([128, T], dt.float32)
            nc.sync.dma_start(iota_sb[:], iota_d[:, :])
            bias_sb = constp.tile([128, C], dt.float32)
            nc.sync.dma_start(bias_sb[:], bias_d[:, :])
            wct_sb = constp.tile([128, C], dt.float32)
            nc.sync.dma_start(wct_sb[:], wct_d[:, :])
            wdt_sb = constp.tile([128, C], dt.float32)
            nc.sync.dma_start(wdt_sb[:], wdt_d[:, :])

            # resident per-core data
            tl_sb = resp.tile([128, ctn], dt.float32)
            nc.sync.dma_start(tl_sb[:], tl_d[:, :])
            idx_sb = resp.tile([128, ctn * 8], dt.int16)
            nc.sync.dma_start(idx_sb[:], idx16_d[:, :])
            g_sb = resp.tile([128, ctn], dt.float32)
            nc.sync.dma_start(g_sb[:], degs_d[:, :])
            nc.vector.reciprocal(g_sb[:], g_sb[:])
            nc.scalar.activation(g_sb[:], g_sb[:], act.Sqrt)
            f_sb = [resp.tile([128, ctn], dt.float32, tag=f"f{b}", name=f"f{b}")
                    for b in range(2)]

            # dis for own target nodes
            disown_sb = resp.tile([128, ntc], dt.float32)
            nc.sync.dma_start(disown_sb[:], degown_d[:, :])
            nc.vector.reciprocal(disown_sb[:], disown_sb[:])
            nc.scalar.activation(disown_sb[:], disown_sb[:], act.Sqrt)

            # ---- prepass: f0/f1 from flux/fdo ----
            with tc.tile_pool(name="pp", bufs=1) as ppp:
                fdo_sb = ppp.tile([128, ctn], dt.float32)
                nc.sync.dma_start(fdo_sb[:], fdo_d[:, :])
                c1 = ppp.tile([128, ctn], dt.float32)
                nc.vector.tensor_scalar(
                    c1[:], fdo_sb[:], 2.0, -1.0, alu.mult, alu.add)
                c0 = ppp.tile([128, ctn], dt.float32)
                nc.vector.tensor_scalar(
                    c0[:], fdo_sb[:], -1.0, 1.0, alu.mult, alu.add)
                for b, (fsd, ftd) in enumerate(((fs0_d, ft0_d), (fs1_d, ft1_d))):
                    fs_sb = ppp.tile([128, ctn], dt.float32, tag="fs")
                    nc.sync.dma_start(fs_sb[:], fsd[:, :])
                    ft_sb = ppp.tile([128, ctn], dt.float32, tag="ft")
                    nc.sync.dma_start(ft_sb[:], ftd[:, :])
                    nc.vector.tensor_mul(fs_sb[:], fs_sb[:], ft_sb[:])
                    nc.scalar.activation(
                        ft_sb[:], fs_sb[:], act.Sigmoid, scale=2.0)
                    nc.vector.tensor_mul(f_sb[b][:], ft_sb[:], c1[:])
                    nc.vector.tensor_add(f_sb[b][:], f_sb[b][:], c0[:])

            # ---- main loop over node tiles ----
            with (
                tc.tile_pool(name="xg", bufs=3) as xgp,
                tc.tile_pool(name="wv", bufs=2) as wvp,
                tc.tile_pool(name="oe", bufs=2) as oep,
                tc.tile_pool(name="og", bufs=6) as ogp,
                tc.tile_pool(name="uv", bufs=2) as uvp,
                tc.tile_pool(name="outp", bufs=2) as outsp,
                tc.tile_pool(name="ps_tv", bufs=2, space="PSUM") as pstv,
                tc.tile_pool(name="ps_o", bufs=2, space="PSUM") as pso,
            ):
                for tt in range(ntc):
                    xga = xgp.tile([128, capa * 2 * C], dt.float32, tag="xga")
                    nc.gpsimd.dma_gather(
                        xga[:].rearrange("p (c r) -> p c r", r=2 * C),
                        xpa[:, :],
                        idx_sb[:, tt * ct * 8: tt * ct * 8 + capa * 8],
                        capa * CHUNK, capa * CHUNK, 2 * C,
                        single_packet=False, queue_num=(2 * tt) % 4,
                    )
                    xgb = xgp.tile([128, capb * 2 * C], dt.float32, tag="xgb")
                    nc.gpsimd.dma_gather(
                        xgb[:].rearrange("p (c r) -> p c r", r=2 * C),
                        xpb[:, :],
                        idx_sb[:, tt * ct * 8 + capa * 8:(tt + 1) * ct * 8],
                        capb * CHUNK, capb * CHUNK, 2 * C,
                        single_packet=False, queue_num=(2 * tt + 1) % 4,
                    )

                    # merged 0/1 one-hot for all chunks of this tile
                    o_all = oep.tile([128, ct * T], dt.float32, tag="oe")
                    tl_cols = tl_sb[:, tt * ct:(tt + 1) * ct].unsqueeze(2)
                    nc.vector.tensor_tensor(
                        o_all[:].rearrange("p (c t) -> p c t", t=T),
                        tl_cols.to_broadcast([128, ct, T]),
                        iota_sb[:].unsqueeze(1).to_broadcast([128, ct, T]),
                        alu.is_equal,
                    )

                    # f-scaled V weights, merged per segment & batch-half
                    wva = wvp.tile([128, capa * 2 * C], dt.float32, tag="wva")
                    wvb = wvp.tile([128, capb * 2 * C], dt.float32, tag="wvb")
                    for (w3, x3, nch, foff) in (
                        (wva, xga, capa, tt * ct),
                        (wvb, xgb, capb, tt * ct + capa),
                    ):
                        wv3 = w3[:].rearrange("p (c r) -> p c r", r=2 * C)
                        xg3 = x3[:].rearrange("p (c r) -> p c r", r=2 * C)
                        for bi in range(2):
                            fcols = f_sb[bi][:, foff:foff + nch]
                            nc.vector.tensor_tensor(
                                wv3[:, :, bi * C:(bi + 1) * C],
                                xg3[:, :, bi * C:(bi + 1) * C],
                                fcols.unsqueeze(2).to_broadcast([128, nch, C]),
                                alu.mult,
                            )

                    t_ps = pstv.tile([128, T], dt.float32, tag="t_ps")
                    v_ps = pstv.tile([128, T], dt.float32, tag="v_ps")
                    for c in range(ct):
                        # dis_src-scaled one-hot on the (otherwise idle) ACT
                        og2 = ogp.tile([128, T], dt.float32, tag="og")
                        nc.scalar.activation(
                            og2[:], o_all[:, c * T:(c + 1) * T], act.Copy,
                            scale=g_sb[:, tt * ct + c: tt * ct + c + 1])
                        if c < capa:
                            xsl = xga[:, c * 2 * C:(c + 1) * 2 * C]
                            wsl = wva[:, c * 2 * C:(c + 1) * 2 * C]
                        else:
                            cc = c - capa
                            xsl = xgb[:, cc * 2 * C:(cc + 1) * 2 * C]
                            wsl = wvb[:, cc * 2 * C:(cc + 1) * 2 * C]
                        nc.tensor.matmul(
                            out=t_ps[:], lhsT=xsl, rhs=og2[:],
                            start=(c == 0), stop=(c == ct - 1),
                        )
                        nc.tensor.matmul(
                            out=v_ps[:], lhsT=wsl, rhs=og2[:],
                            start=(c == 0), stop=(c == ct - 1),
                        )

                    # epilogue
                    vm = uvp.tile([128, T], dt.float32, tag="vm")
                    nc.vector.tensor_copy(out=vm[:], in_=v_ps[:])
                    um = uvp.tile([128, T], dt.float32, tag="um")
                    nc.vector.tensor_tensor(um[:], t_ps[:], vm[:], alu.subtract)

                    for bi in range(2):
                        rows = slice(64 * bi, 64 * bi + 64)
                        op_ps = pso.tile([T, C], dt.float32, tag=f"op{bi}")
                        nc.tensor.matmul(
                            out=op_ps[:], lhsT=um[rows, :], rhs=wct_sb[rows, :],
                            start=True, stop=False,
                        )
                        nc.tensor.matmul(
                            out=op_ps[:], lhsT=vm[rows, :], rhs=wdt_sb[rows, :],
                            start=False, stop=True,
                        )
                        o_sb = outsp.tile([128, C], dt.float32, tag=f"os{bi}")
                        nc.vector.tensor_scalar(
                            o_sb[:T, :], op_ps[:], disown_sb[:T, tt:tt + 1],
                            None, alu.mult)
                        nc.vector.tensor_add(
                            o_sb[:T, :], o_sb[:T, :], bias_sb[:T, :])
                        nc.sync.dma_start(
                            outs[bi][tt * T:(tt + 1) * T, :], o_sb[:T, :])

    nc.compile()
    return nc


def _shared_weights(W_conc, W_disc, bias):
    wct2 = np.zeros((128, C), np.float32)
    wdt2 = np.zeros((128, C), np.float32)
    wct2[:64] = np.asarray(W_conc, np.float32).T  # WcT[i, o] = Wc[o, i]
    wct2[64:] = wct2[:64]
    wdt2[:64] = np.asarray(W_disc, np.float32).T
    wdt2[64:] = wdt2[:64]
    biasr = np.tile(np.asarray(bias, np.float32)[None, :], (128, 1))
    return wct2, wdt2, biasr


_NC_CACHE = {}


def _caps_needed(edge_index, n, n_cores, tile, split):
    """Max per-tile chunk counts for the A/B table split (self loops incl.)."""
    src0 = np.asarray(edge_index[0]).astype(np.int64)
    tgt0 = np.asarray(edge_index[1]).astype(np.int64)
    loops = np.arange(n, dtype=np.int64)
    src_all = np.concatenate([src0, loops])
    tgt_all = np.concatenate([tgt0, loops])
    order = np.argsort(tgt_all, kind="stable")
    tgt_s, src_s = tgt_all[order], src_all[order]
    starts = np.searchsorted(tgt_s, np.arange(0, n + 1, tile))
    na = np.add.reduceat((src_s < split).astype(np.int64), starts[:-1])
    tot = np.diff(starts)
    maxa = int(na.max())
    maxb = int((tot - na).max())
    return -(-maxa // CHUNK), -(-maxb // CHUNK)


def _make_in_maps(x, edge_index, f_disc_orig, fluxes, W_conc, W_disc, bias,
                  cfg):
    shared, cores = prep(x, edge_index, f_disc_orig, fluxes, cfg)
    wct2, wdt2, biasr = _shared_weights(W_conc, W_disc, bias)
    in_maps = []
    for core in range(cfg.n_cores):
        m = dict(shared)
        m.update(cores[core])
        m["wct2"] = wct2
        m["wdt2"] = wdt2
        m["biasr"] = biasr
        in_maps.append(m)
    return in_maps


def _run(inputs, trace=False):
    from concourse.bass_utils import run_bass_kernel_spmd

    x = np.asarray(inputs["x"], np.float32)
    n = x.shape[1]
    capa, capb = _caps_needed(inputs["edge_index"], n, N_CORES, TILE, SPLIT)
    cfg = Cfg(n_nodes=n, n_cores=N_CORES, tile=TILE, split=SPLIT,
              capa=max(capa, 23), capb=max(capb, 13))
    in_maps = _make_in_maps(
        x, inputs["edge_index"], inputs["f_disc_orig"], inputs["fluxes"],
        inputs["W_conc"], inputs["W_disc"], inputs["bias"], cfg)

    if cfg not in _NC_CACHE:
        _NC_CACHE[cfg] = build_nc(cfg)
    nc = _NC_CACHE[cfg]

    res = run_bass_kernel_spmd(nc, in_maps, list(range(cfg.n_cores)),
                               trace=trace)
    out = np.zeros((BATCH, n, C), np.float32)
    npc = cfg.nodes_per_core
    for core in range(cfg.n_cores):
        out[0, core * npc:(core + 1) * npc] = res.results[core]["out0"]
        out[1, core * npc:(core + 1) * npc] = res.results[core]["out1"]
    return out, res


def kernel(x, edge_index, f_disc_orig, fluxes, W_conc, W_disc, bias):
    out, _ = _run(dict(x=x, edge_index=edge_index, f_disc_orig=f_disc_orig,
                       fluxes=fluxes, W_conc=W_conc, W_disc=W_disc, bias=bias))
    return out


def profile_run(inputs):
    out, res = _run(inputs, trace=True)
    return res.exec_time_ns


# revision 17
# speedup vs baseline: 2.6638x; 1.4290x over previous
"""Trainium2 Bass kernel for nn_ReaReaConv (GCN-style message passing with
dynamic edge gating).

Math (per batch b):
    deg[n]   = in-degree(n) + 1 (self loop);  dis = rsqrt(deg)
    f_e      = keep*fdo + (1-keep)*(1-fdo), keep = sigmoid(2*flux[src]*flux[tgt])
    out[t]   = dis_t * ( (T-V)[t] @ Wc^T + V[t] @ Wd^T ) + bias
    T[t]     = sum_{e->t} dis_src * x[src_e]          (self loop: f=0 edge)
    V[t]     = sum_{e->t} dis_src * f_e * x[src_e]

Sharding: each of the 8 cores owns N/8 target nodes (tiles of 125). Host sorts
edges by target tile (indices/layout only; all FP math runs on device).

Device phases:
 1. dis = rsqrt(deg) densely; build xp[n] = [dis_n*x[0,n], dis_n*x[1,n]]
    (the dis_src-prescaled gather table, 512B rows, both batches).
 2. Whole-core prepass computes per-edge f0/f1 from flux/fdo metadata.
 3. Per 125-node tile: dma_gather the tile's edges' xp rows (A/B table split
    for int16 indices), one merged is_equal builds all chunk one-hots, merged
    multiplies build f-scaled V-weights, then 2 PE matmuls per 128-edge chunk
    accumulate T/V in PSUM. Epilogue: U=T-V, project with Wc/Wd, scale by
    dis_tgt, add bias, store densely.
"""

from dataclasses import dataclass

import numpy as np

# -------------------- problem constants --------------------
N_NODES = 50000
N_EDGES = 1600000
BATCH = 2
C = 64
N_CORES = 8
TILE = 125           # target nodes per tile (one-hot width)
CHUNK = 128          # edges per matmul chunk (PE contraction)
SPLIT = 32768        # gather-table split (int16 signed index limit)
SELF_FLUX = 30.0     # sigmoid(2*30*30)==1.0 -> f==0 for self-loop edges


@dataclass(frozen=True)
class Cfg:
    n_nodes: int
    n_cores: int
    tile: int
    split: int
    capa: int  # chunks per tile from table A (src < split)
    capb: int  # chunks per tile from table B

    @property
    def nodes_per_core(self):
        return self.n_nodes // self.n_cores

    @property
    def ntc(self):  # tiles per core
        return self.nodes_per_core // self.tile

    @property
    def ct(self):
        return self.capa + self.capb

    @property
    def ctn(self):
        return self.ntc * self.ct

    @property
    def na(self):
        return min(self.split, self.n_nodes)

    @property
    def nb(self):
        return self.n_nodes - self.na

    @property
    def nblk(self):
        return -(-self.n_nodes // 128)


# -------------------- host prep (indices / layout only) --------------------

def _wrap16(idx_flat):
    """dma_gather index layout: [128, n/16] int16, idx[p, s] = flat[s*16+p],
    replicated across the 8 gpsimd cores (partition blocks of 16)."""
    n = len(idx_flat)
    assert n % 16 == 0
    w = np.asarray(idx_flat, np.int16).reshape(n // 16, 16).T  # [16, n/16]
    return np.tile(w, (8, 1))  # [128, n/16]


def prep(x, edge_index, f_disc_orig, fluxes, cfg: Cfg):
    """Returns (shared dict, list of per-core dicts). Integer/index/layout
    work only — no floating-point arithmetic."""
    n = cfg.n_nodes
    src0 = np.asarray(edge_index[0]).astype(np.int64)
    tgt0 = np.asarray(edge_index[1]).astype(np.int64)
    x = np.asarray(x, np.float32)
    fdo_in = np.asarray(f_disc_orig, np.float32)
    fluxes = np.asarray(fluxes, np.float32)

    deg = (np.bincount(tgt0, minlength=n) + 1).astype(np.float32)  # int-valued

    loops = np.arange(n, dtype=np.int64)
    src_all = np.concatenate([src0, loops])
    tgt_all = np.concatenate([tgt0, loops])
    sf = np.full(n, SELF_FLUX, np.float32)
    per_edge_all = np.stack([
        np.concatenate([fdo_in, np.zeros(n, np.float32)]),
        np.concatenate([fluxes[0][src0], sf]),
        np.concatenate([fluxes[1][src0], sf]),
        np.concatenate([fluxes[0][tgt0], sf]),
        np.concatenate([fluxes[1][tgt0], sf]),
        deg[np.concatenate([src0, loops])],  # deg at src end (int-valued)
    ])  # [6, E+N]: fdo, fs0, fs1, ft0, ft1, degs

    perm = np.argsort(tgt_all, kind="stable")
    src_s = src_all[perm]
    tgt_s = tgt_all[perm]
    per_edge = per_edge_all[:, perm]

    tile_starts = np.searchsorted(tgt_s, np.arange(0, n + 1, cfg.tile))
    is_a = src_s < cfg.split

    ct, ntc, ctn = cfg.ct, cfg.ntc, cfg.ctn

    shared = {
        # gather tables: row n = [x[0,n,:], x[1,n,:]]  (pure interleave)
        "xpa": np.ascontiguousarray(
            np.concatenate([x[0, : cfg.na], x[1, : cfg.na]], axis=1)),
        "xpb": np.ascontiguousarray(
            np.concatenate([x[0, cfg.na:], x[1, cfg.na:]], axis=1)),
        "iota": np.tile(np.arange(cfg.tile, dtype=np.float32), (128, 1)),
    }

    names = ["fdo", "fs0", "fs1", "ft0", "ft1", "degs"]
    cores = []
    for core in range(cfg.n_cores):
        tl_all = np.full((128, ctn), -1.0, np.float32)
        pe_all = np.zeros((6, 128, ctn), np.float32)
        pe_all[5] = 1.0  # pad deg_src = 1
        idx16 = np.zeros((128, ctn * 8), np.int16)
        degown = np.ones((128, ntc), np.float32)
        for tt in range(ntc):
            t = core * ntc + tt
            t0 = t * cfg.tile
            s, e = tile_starts[t], tile_starts[t + 1]
            sel_a = np.nonzero(is_a[s:e])[0] + s
            sel_b = np.nonzero(~is_a[s:e])[0] + s
            nA, nB = len(sel_a), len(sel_b)
            assert nA <= cfg.capa * CHUNK, (t, nA, cfg.capa * CHUNK)
            assert nB <= cfg.capb * CHUNK, (t, nB, cfg.capb * CHUNK)

            ids = np.zeros(ct * CHUNK, np.int64)
            tl = np.full(ct * CHUNK, -1.0, np.float32)
            pe = np.zeros((6, ct * CHUNK), np.float32)
            pe[5] = 1.0
            off = cfg.capa * CHUNK
            ids[:nA] = src_s[sel_a]
            ids[off:off + nB] = src_s[sel_b] - cfg.na
            tl[:nA] = tgt_s[sel_a] - t0
            tl[off:off + nB] = tgt_s[sel_b] - t0
            pe[:, :nA] = per_edge[:, sel_a]
            pe[:, off:off + nB] = per_edge[:, sel_b]

            # chunk-transposed layout: element (p, c) = edge[c*128+p]
            cols = slice(tt * ct, (tt + 1) * ct)
            tl_all[:, cols] = tl.reshape(ct, CHUNK).T
            for j in range(6):
                pe_all[j][:, cols] = pe[j].reshape(ct, CHUNK).T

            islice = idx16[:, tt * ct * 8:(tt + 1) * ct * 8]
            islice[:, : cfg.capa * 8] = _wrap16(ids[:off])
            islice[:, cfg.capa * 8:] = _wrap16(ids[off:])

            degown[:cfg.tile, tt] = deg[t0:t0 + cfg.tile]
        d = {"tl": tl_all, "idx16": idx16, "degown": degown}
        for j, nm in enumerate(names):
            d[nm] = np.ascontiguousarray(pe_all[j])
        cores.append(d)
    return shared, cores


# -------------------- device program --------------------

def build_nc(cfg: Cfg):
    import concourse.bass as bass
    import concourse.tile as tile
    from concourse import bacc, mybir

    dt = mybir.dt
    act = mybir.ActivationFunctionType
    alu = mybir.AluOpType

    ct, capa, capb = cfg.ct, cfg.capa, cfg.capb
    ntc, T, ctn = cfg.ntc, cfg.tile, cfg.ctn
    n = cfg.n_nodes

    nc = bacc.Bacc("TRN2", target_bir_lowering=False, debug=False,
                   num_swdge_queues=4)

    xpa = nc.dram_tensor("xpa", [cfg.na, 2 * C], dt.float32, kind="ExternalInput")
    xpb = nc.dram_tensor("xpb", [cfg.nb, 2 * C], dt.float32, kind="ExternalInput")
    tl_d = nc.dram_tensor("tl", [128, ctn], dt.float32, kind="ExternalInput")
    fdo_d = nc.dram_tensor("fdo", [128, ctn], dt.float32, kind="ExternalInput")
    fs0_d = nc.dram_tensor("fs0", [128, ctn], dt.float32, kind="ExternalInput")
    fs1_d = nc.dram_tensor("fs1", [128, ctn], dt.float32, kind="ExternalInput")
    ft0_d = nc.dram_tensor("ft0", [128, ctn], dt.float32, kind="ExternalInput")
    ft1_d = nc.dram_tensor("ft1", [128, ctn], dt.float32, kind="ExternalInput")
    degs_d = nc.dram_tensor("degs", [128, ctn], dt.float32, kind="ExternalInput")
    idx16_d = nc.dram_tensor("idx16", [128, ctn * 8], dt.int16, kind="ExternalInput")
    degown_d = nc.dram_tensor("degown", [128, ntc], dt.float32, kind="ExternalInput")
    iota_d = nc.dram_tensor("iota", [128, T], dt.float32, kind="ExternalInput")
    wct_d = nc.dram_tensor("wct2", [128, C], dt.float32, kind="ExternalInput")
    wdt_d = nc.dram_tensor("wdt2", [128, C], dt.float32, kind="ExternalInput")
    bias_d = nc.dram_tensor("biasr", [128, C], dt.float32, kind="ExternalInput")
    out0 = nc.dram_tensor("out0", [ntc * T, C], dt.float32, kind="ExternalOutput")
    out1 = nc.dram_tensor("out1", [ntc * T, C], dt.float32, kind="ExternalOutput")
    outs = [out0, out1]

    with tile.TileContext(nc) as tc:
        with (
            tc.tile_pool(name="const", bufs=1) as constp,
            tc.tile_pool(name="res", bufs=1) as resp,
        ):
            iota_sb = constp.tile([128, T], dt.float32)
            nc.sync.dma_start(iota_sb[:], iota_d[:, :])
            bias_sb = constp.tile([128, C], dt.float32)
            nc.sync.dma_start(bias_sb[:], bias_d[:, :])
            wct_sb = constp.tile([128, C], dt.float32)
            nc.sync.dma_start(wct_sb[:], wct_d[:, :])
            wdt_sb = constp.tile([128, C], dt.float32)
            nc.sync.dma_start(wdt_sb[:], wdt_d[:, :])

            # resident per-core data
            tl_sb = resp.tile([128, ctn], dt.float32)
            nc.sync.dma_start(tl_sb[:], tl_d[:, :])
            idx_sb = resp.tile([128, ctn * 8], dt.int16)
            nc.sync.dma_start(idx_sb[:], idx16_d[:, :])
            g_sb = resp.tile([128, ctn], dt.float32)
            nc.sync.dma_start(g_sb[:], degs_d[:, :])
            nc.vector.reciprocal(g_sb[:], g_sb[:])
            nc.scalar.activation(g_sb[:], g_sb[:], act.Sqrt)
            f_sb = [resp.tile([128, ctn], dt.float32, tag=f"f{b}", name=f"f{b}")
                    for b in range(2)]

            # dis for own target nodes
            disown_sb = resp.tile([128, ntc], dt.float32)
            nc.sync.dma_start(disown_sb[:], degown_d[:, :])
            nc.vector.reciprocal(disown_sb[:], disown_sb[:])
            nc.scalar.activation(disown_sb[:], disown_sb[:], act.Sqrt)

            # ---- prepass: f0/f1 from flux/fdo ----
            with tc.tile_pool(name="pp", bufs=1) as ppp:
                fdo_sb = ppp.tile([128, ctn], dt.float32)
                nc.sync.dma_start(fdo_sb[:], fdo_d[:, :])
                c1 = ppp.tile([128, ctn], dt.float32)
                nc.vector.tensor_scalar(
                    c1[:], fdo_sb[:], 2.0, -1.0, alu.mult, alu.add)
                c0 = ppp.tile([128, ctn], dt.float32)
                nc.vector.tensor_scalar(
                    c0[:], fdo_sb[:], -1.0, 1.0, alu.mult, alu.add)
                for b, (fsd, ftd) in enumerate(((fs0_d, ft0_d), (fs1_d, ft1_d))):
                    fs_sb = ppp.tile([128, ctn], dt.float32, tag="fs")
                    nc.sync.dma_start(fs_sb[:], fsd[:, :])
                    ft_sb = ppp.tile([128, ctn], dt.float32, tag="ft")
                    nc.sync.dma_start(ft_sb[:], ftd[:, :])
                    nc.vector.tensor_mul(fs_sb[:], fs_sb[:], ft_sb[:])
                    nc.scalar.activation(
                        ft_sb[:], fs_sb[:], act.Sigmoid, scale=2.0)
                    nc.vector.tensor_mul(f_sb[b][:], ft_sb[:], c1[:])
                    nc.vector.tensor_add(f_sb[b][:], f_sb[b][:], c0[:])

            # ---- main loop over node tiles ----
            with (
                tc.tile_pool(name="xg", bufs=3) as xgp,
                tc.tile_pool(name="wv", bufs=2) as wvp,
                tc.tile_pool(name="oe", bufs=2) as oep,
                tc.tile_pool(name="uv", bufs=2) as uvp,
                tc.tile_pool(name="outp", bufs=2) as outsp,
                tc.tile_pool(name="ps_tv", bufs=2, space="PSUM") as pstv,
                tc.tile_pool(name="ps_o", bufs=2, space="PSUM") as pso,
            ):
                qrr = 0
                for tt in range(ntc):
                    ib = tt * ct * 8
                    xga = xgp.tile([128, capa * 2 * C], dt.float32, tag="xga")
                    xga3 = xga[:].rearrange("p (c r) -> p c r", r=2 * C)
                    ca1 = capa // 2
                    for (c0_, c1_) in ((0, ca1), (ca1, capa)):
                        nch = c1_ - c0_
                        nc.gpsimd.dma_gather(
                            xga3[:, c0_:c1_],
                            xpa[:, :],
                            idx_sb[:, ib + c0_ * 8: ib + c1_ * 8],
                            nch * CHUNK, nch * CHUNK, 2 * C,
                            single_packet=False, queue_num=qrr % 4,
                        )
                        qrr += 1
                    xgb = xgp.tile([128, capb * 2 * C], dt.float32, tag="xgb")
                    nc.gpsimd.dma_gather(
                        xgb[:].rearrange("p (c r) -> p c r", r=2 * C),
                        xpb[:, :],
                        idx_sb[:, ib + capa * 8:(tt + 1) * ct * 8],
                        capb * CHUNK, capb * CHUNK, 2 * C,
                        single_packet=False, queue_num=qrr % 4,
                    )
                    qrr += 1

                    # merged one-hot for all chunks, then dis_src scale in place
                    o_all = oep.tile([128, ct * T], dt.float32, tag="oe")
                    o3 = o_all[:].rearrange("p (c t) -> p c t", t=T)
                    tl_cols = tl_sb[:, tt * ct:(tt + 1) * ct].unsqueeze(2)
                    nc.vector.tensor_tensor(
                        o3,
                        tl_cols.to_broadcast([128, ct, T]),
                        iota_sb[:].unsqueeze(1).to_broadcast([128, ct, T]),
                        alu.is_equal,
                    )
                    g_cols = g_sb[:, tt * ct:(tt + 1) * ct].unsqueeze(2)
                    nc.vector.tensor_tensor(
                        o3, o3, g_cols.to_broadcast([128, ct, T]), alu.mult)

                    # f-scaled V weights: batch1 merged on DVE, batch0 per
                    # chunk on the otherwise-idle ACT (scale is per-partition)
                    wva = wvp.tile([128, capa * 2 * C], dt.float32, tag="wva")
                    wvb = wvp.tile([128, capb * 2 * C], dt.float32, tag="wvb")
                    for (w3, x3, nch, foff) in (
                        (wva, xga, capa, tt * ct),
                        (wvb, xgb, capb, tt * ct + capa),
                    ):
                        wv3 = w3[:].rearrange("p (c r) -> p c r", r=2 * C)
                        xg3 = x3[:].rearrange("p (c r) -> p c r", r=2 * C)
                        fcols = f_sb[1][:, foff:foff + nch]
                        nc.vector.tensor_tensor(
                            wv3[:, :, C:2 * C],
                            xg3[:, :, C:2 * C],
                            fcols.unsqueeze(2).to_broadcast([128, nch, C]),
                            alu.mult,
                        )
                        for cc in range(nch):
                            nc.scalar.activation(
                                wv3[:, cc, 0:C], xg3[:, cc, 0:C], act.Copy,
                                scale=f_sb[0][:, foff + cc: foff + cc + 1])

                    t_ps = pstv.tile([128, T], dt.float32, tag="t_ps")
                    v_ps = pstv.tile([128, T], dt.float32, tag="v_ps")
                    for c in range(ct):
                        og2 = o_all[:, c * T:(c + 1) * T]
                        if c < capa:
                            xsl = xga[:, c * 2 * C:(c + 1) * 2 * C]
                            wsl = wva[:, c * 2 * C:(c + 1) * 2 * C]
                        else:
                            cc = c - capa
                            xsl = xgb[:, cc * 2 * C:(cc + 1) * 2 * C]
                            wsl = wvb[:, cc * 2 * C:(cc + 1) * 2 * C]
                        nc.tensor.matmul(
                            out=t_ps[:], lhsT=xsl, rhs=og2,
                            start=(c == 0), stop=(c == ct - 1),
                        )
                        nc.tensor.matmul(
                            out=v_ps[:], lhsT=wsl, rhs=og2,
                            start=(c == 0), stop=(c == ct - 1),
                        )

                    # epilogue
                    vm = uvp.tile([128, T], dt.float32, tag="vm")
                    nc.vector.tensor_copy(out=vm[:], in_=v_ps[:])
                    um = uvp.tile([128, T], dt.float32, tag="um")
                    nc.vector.tensor_tensor(um[:], t_ps[:], vm[:], alu.subtract)

                    for bi in range(2):
                        rows = slice(64 * bi, 64 * bi + 64)
                        op_ps = pso.tile([T, C], dt.float32, tag=f"op{bi}")
                        nc.tensor.matmul(
                            out=op_ps[:], lhsT=um[rows, :], rhs=wct_sb[rows, :],
                            start=True, stop=False,
                        )
                        nc.tensor.matmul(
                            out=op_ps[:], lhsT=vm[rows, :], rhs=wdt_sb[rows, :],
                            start=False, stop=True,
                        )
                        o_sb = outsp.tile([128, C], dt.float32, tag=f"os{bi}")
                        nc.vector.tensor_scalar(
                            o_sb[:T, :], op_ps[:], disown_sb[:T, tt:tt + 1],
                            None, alu.mult)
                        nc.vector.tensor_add(
                            o_sb[:T, :], o_sb[:T, :], bias_sb[:T, :])
                        nc.sync.dma_start(
                            outs[bi][tt * T:(tt + 1) * T, :], o_sb[:T, :])

    nc.compile()
    return nc


def _shared_weights(W_conc, W_disc, bias):
    wct2 = np.zeros((128, C), np.float32)
    wdt2 = np.zeros((128, C), np.float32)
    wct2[:64] = np.asarray(W_conc, np.float32).T  # WcT[i, o] = Wc[o, i]
    wct2[64:] = wct2[:64]
    wdt2[:64] = np.asarray(W_disc, np.float32).T
    wdt2[64:] = wdt2[:64]
    biasr = np.tile(np.asarray(bias, np.float32)[None, :], (128, 1))
    return wct2, wdt2, biasr


_NC_CACHE = {}


def _caps_needed(edge_index, n, n_cores, tile, split):
    """Max per-tile chunk counts for the A/B table split (self loops incl.)."""
    src0 = np.asarray(edge_index[0]).astype(np.int64)
    tgt0 = np.asarray(edge_index[1]).astype(np.int64)
    loops = np.arange(n, dtype=np.int64)
    src_all = np.concatenate([src0, loops])
    tgt_all = np.concatenate([tgt0, loops])
    order = np.argsort(tgt_all, kind="stable")
    tgt_s, src_s = tgt_all[order], src_all[order]
    starts = np.searchsorted(tgt_s, np.arange(0, n + 1, tile))
    na = np.add.reduceat((src_s < split).astype(np.int64), starts[:-1])
    tot = np.diff(starts)
    maxa = int(na.max())
    maxb = int((tot - na).max())
    return -(-maxa // CHUNK), -(-maxb // CHUNK)


def _make_in_maps(x, edge_index, f_disc_orig, fluxes, W_conc, W_disc, bias,
                  cfg):
    shared, cores = prep(x, edge_index, f_disc_orig, fluxes, cfg)
    wct2, wdt2, biasr = _shared_weights(W_conc, W_disc, bias)
    in_maps = []
    for core in range(cfg.n_cores):
        m = dict(shared)
        m.update(cores[core])
        m["wct2"] = wct2
        m["wdt2"] = wdt2
        m["biasr"] = biasr
        in_maps.append(m)
    return in_maps


def _run(inputs, trace=False):
    from concourse.bass_utils import run_bass_kernel_spmd

    x = np.asarray(inputs["x"], np.float32)
    n = x.shape[1]
    capa, capb = _caps_needed(inputs["edge_index"], n, N_CORES, TILE, SPLIT)
    cfg = Cfg(n_nodes=n, n_cores=N_CORES, tile=TILE, split=SPLIT,
              capa=max(capa, 23), capb=max(capb, 13))
    in_maps = _make_in_maps(
        x, inputs["edge_index"], inputs["f_disc_orig"], inputs["fluxes"],
        inputs["W_conc"], inputs["W_disc"], inputs["bias"], cfg)

    if cfg not in _NC_CACHE:
        _NC_CACHE[cfg] = build_nc(cfg)
    nc = _NC_CACHE[cfg]

    res = run_bass_kernel_spmd(nc, in_maps, list(range(cfg.n_cores)),
                               trace=trace)
    out = np.zeros((BATCH, n, C), np.float32)
    npc = cfg.nodes_per_core
    for core in range(cfg.n_cores):
        out[0, core * npc:(core + 1) * npc] = res.results[core]["out0"]
        out[1, core * npc:(core + 1) * npc] = res.results[core]["out1"]
    return out, res


def kernel(x, edge_index, f_disc_orig, fluxes, W_conc, W_disc, bias):
    out, _ = _run(dict(x=x, edge_index=edge_index, f_disc_orig=f_disc_orig,
                       fluxes=fluxes, W_conc=W_conc, W_disc=W_disc, bias=bias))
    return out


def profile_run(inputs):
    out, res = _run(inputs, trace=True)
    return res.exec_time_ns
